# revision 1
# baseline (speedup 1.0000x reference)
"""Trainium2 Bass kernel for nn_CrossAttention (B=4, C=256, H=W=64).

Sharding: 8 cores = (batch b, query-half h). Each core computes, for its
batch and its half of the query rows i (IH=2048):
  q = Wq x_i + bq        [32, 2048] stored 4x row-replicated as q4 [128, 2048]
  k = Wk x_f             [32, 4096] stored 4x row-replicated as k4 [128, 4096]
                         (bk dropped: constant-in-j shift is softmax-invariant)
  vT = (Wv x_f)^T        [4096, 256] bf16  (bv folded into bc_eff on host)
  S^T[j, i] = k_j . q_i  (transposed: softmax denom + attended matmul need no
                          on-chip transposes; row-replication lets two K=32
                          score matmuls run concurrently in PE row strips)
  E = exp(S^T) bf16      (no max subtraction: |S| <~ 30, exp safe in f32)
  r[i] = sum_j E[j, i]   (ones-matmuls col-packed 4-per-slot at M=1)
  att[c, i] = sum_j vT[j, c] E[j, i] / r[i]
  comb = Wc [x_i; att] + bc_eff ; out[i] = sum_c |comb|  (ones-matmul)
Structure: query blocks processed in pairs (ib_a, ib_b) so each attended
weight load serves 2 matmuls; PSUM: 4 att accумulators + 2 score staging +
1 r bank + 1 scratch = 8 banks, PE never waits on exp (keeps HAM at 2.4GHz).
"""

import numpy as np
import ml_dtypes

import concourse.bass as bass
import concourse.bacc as bacc
import concourse.tile as tile
import concourse.mybir as mybir
from concourse.bass_utils import run_bass_kernel_spmd

B, C, HH, WW = 4, 256, 64, 64
N = HH * WW          # 4096
CQK = 32
IH = N // 2          # 2048 query rows per core
NCORES = 8
NJC = N // 128       # 32 key-dim 128-chunks
NG = NJC // 2        # 16 groups of 2 key-chunks
NPAIR = IH // 1024   # 2 query-block pairs per branch

F32 = mybir.dt.float32
F32R = mybir.dt.float32r
BF16 = mybir.dt.bfloat16
AF = mybir.ActivationFunctionType


def _bcast_ap(ap, p):
    """0-stride partition-broadcast view of a [1, n] DRAM AP (for DMA)."""
    return bass.AP(tensor=ap.tensor, offset=ap.offset,
                   ap=[[0, p]] + [list(d) for d in ap.ap[1:]])


def build_program(nc, tc):
    # ---- DRAM I/O ------------------------------------------------------
    dram = {}
    for name, shape, dt in [
        ("x1f", [2, 128, N], F32R), ("x2f", [2, 128, N], F32R),
        ("x1i", [2, 128, IH], F32R), ("x2i", [2, 128, IH], F32R),
        ("wqt", [2, 128, 128], F32R), ("wkt", [2, 128, 128], F32R),
        ("wvt", [2, 128, C], F32R),
        ("wctx", [2, 128, C], F32R), ("wcta", [2, 128, C], BF16),
        ("bq", [128, 1], F32), ("bce", [128, 2], F32),
    ]:
        dram[name] = nc.dram_tensor(name, shape, dt, kind="ExternalInput").ap()
    out_d = nc.dram_tensor("out", [2, IH], F32, kind="ExternalOutput").ap()

    import contextlib
    with contextlib.ExitStack() as ctx:
        persist = ctx.enter_context(tc.tile_pool(name="persist", bufs=1))

        wq_sb = persist.tile([128, 2, 128], F32R, tag="wq")
        wk_sb = persist.tile([128, 2, 128], F32R, tag="wk")
        wv_sb = persist.tile([128, 2, C], F32R, tag="wv")
        wcx_sb = persist.tile([128, 2, C], F32R, tag="wcx")
        wca_sb = persist.tile([128, 2, C], BF16, tag="wca")
        bq_sb = persist.tile([128, 1], F32, tag="bq")
        bce_sb = persist.tile([128, 2], F32, tag="bce")
        ones_bf = persist.tile([128, 1], BF16, tag="ones")

        for w, t in [("wqt", wq_sb), ("wkt", wk_sb), ("wvt", wv_sb),
                     ("wctx", wcx_sb), ("wcta", wca_sb)]:
            for kc in range(2):
                nc.sync.dma_start(out=t[:, kc, :], in_=dram[w][kc])
        nc.sync.dma_start(out=bq_sb, in_=dram["bq"])
        nc.sync.dma_start(out=bce_sb, in_=dram["bce"])
        nc.vector.memset(ones_bf, 1.0)

        x1i_sb = [persist.tile([128, IH], F32R, tag=f"x1i{kc}",
                               name=f"x1i{kc}") for kc in range(2)]
        for kc in range(2):
            nc.sync.dma_start(out=x1i_sb[kc], in_=dram["x1i"][kc])

        # projection outputs; k4/vT split in j-halves for earlier consumption
        q4_sb = [persist.tile([128, IH], F32R, tag=f"q{i}", name=f"q{i}")
                 for i in range(2)]
        k4_sb = [[persist.tile([128, N // 2], F32R, tag=f"k{i}{h}",
                               name=f"k{i}{h}") for h in range(2)]
                 for i in range(2)]
        vT_sb = [[persist.tile([128, (NJC // 2) * C], BF16, tag=f"vt{i}{h}",
                               name=f"vt{i}{h}") for h in range(2)]
                 for i in range(2)]
        att_sb = [[persist.tile([128, IH], BF16, tag=f"att{br}{c2}",
                                name=f"att{br}{c2}") for c2 in range(2)]
                  for br in range(2)]

        # ---- phase 1: projections -------------------------------------
        with tc.tile_pool(name="proj_sb", bufs=2) as proj_sb, \
             tc.tile_pool(name="ps_kq", bufs=3, space="PSUM") as ps_kq, \
             tc.tile_pool(name="ps_vt", bufs=2, space="PSUM") as ps_vt:

            # q4 projections (from islice inputs; bq folded via ACT bias)
            for xi in range(2):
                if xi == 0:
                    xi_sb = x1i_sb
                else:
                    xi_sb = [proj_sb.tile([128, IH], F32R, tag="x2i",
                                          name="x2i") for _ in range(2)]
                    for kc in range(2):
                        nc.sync.dma_start(out=xi_sb[kc], in_=dram["x2i"][kc])
                for ib in range(4):
                    sl = bass.ts(ib, 512)
                    qp = ps_kq.tile([128, 512], F32, tag="kq", name="qp")
                    for kc in range(2):
                        nc.tensor.matmul(qp, wq_sb[:, kc, :], xi_sb[kc][:, sl],
                                         start=(kc == 0), stop=(kc == 1))
                    nc.scalar.activation(q4_sb[xi][:, sl], qp, AF.Identity,
                                         bias=bq_sb)

            # k4 and vT projections, x2 first (branch 0 needs vT2)
            for xi, xf_name in [(1, "x2f"), (0, "x1f")]:
                for jh in range(2):
                    xf_t = proj_sb.tile([128, 2, IH], F32R, tag="xf",
                                        name="xf")
                    for kc in range(2):
                        nc.sync.dma_start(
                            out=xf_t[:, kc, :],
                            in_=dram[xf_name][kc][:, jh * IH:(jh + 1) * IH])
                    for jb in range(4):
                        sl = bass.ts(jb, 512)
                        kp = ps_kq.tile([128, 512], F32, tag="kq", name="kp")
                        for kc in range(2):
                            nc.tensor.matmul(kp, wk_sb[:, kc, :],
                                             xf_t[:, kc, sl],
                                             start=(kc == 0), stop=(kc == 1))
                        nc.scalar.activation(k4_sb[xi][jh][:, sl], kp, AF.Copy)
                    for g in range(4):
                        vtp = ps_vt.tile([128, 4, C], F32, tag="vt",
                                         name="vtp")
                        for s in range(4):
                            jsub = g * 4 + s
                            for kc in range(2):
                                nc.tensor.matmul(
                                    vtp[:, s, :],
                                    xf_t[:, kc, bass.ts(jsub, 128)],
                                    wv_sb[:, kc, :],
                                    start=(kc == 0), stop=(kc == 1))
                        nc.vector.tensor_copy(
                            vT_sb[xi][jh][:, bass.ds(g * 4 * C, 4 * C)],
                            vtp.rearrange("p a c -> p (a c)"))

        # ---- phase 2+3: attention with fused combine ------------------
        with tc.tile_pool(name="attn_sb", bufs=1) as attn_sb, \
             tc.tile_pool(name="attn_dram", bufs=2, space="DRAM") as attn_dram, \
             tc.tile_pool(name="ps_att", bufs=1, space="PSUM") as ps_att, \
             tc.tile_pool(name="ps_st", bufs=1, space="PSUM") as ps_st, \
             tc.tile_pool(name="ps_r", bufs=1, space="PSUM") as ps_r:

            for br in range(2):
                q4, k4, vT = q4_sb[br], k4_sb[br], vT_sb[1 - br]
                for ib in range(4):
                    isl = bass.ts(ib, 512)
                    attp = [ps_att.tile([128, 512], F32, tag="attp",
                                        bufs=3, name=f"attp{c2}")
                            for c2 in range(2)]
                    rp = ps_r.tile([128, 512], F32, tag="rp", name="rp")
                    nc.vector.memset(rp, 0.0)
                    for g in range(NG):
                        jcs = (2 * g, 2 * g + 1)
                        jh = g // (NG // 2)
                        jloc = [jc - jh * (NJC // 2) for jc in jcs]
                        stp = ps_st.tile([128, 2, 512], F32, tag="stp",
                                         bufs=2, name="stp")
                        # score matmuls: 2 row strips run concurrently
                        for t in range(2):
                            nc.tensor.matmul(
                                stp[:, t, :],
                                k4[jh][32 * t:32 * (t + 1),
                                       bass.ts(jloc[t], 128)],
                                q4[32 * t:32 * (t + 1), isl],
                                start=True, stop=True,
                                tile_position=(32 * t, 0))
                        est = attn_sb.tile([128, 2, 512], BF16,
                                           tag="est", bufs=6, name="est")
                        nc.scalar.activation(
                            est.rearrange("p a n -> p (a n)"),
                            stp.rearrange("p a n -> p (a n)"), AF.Exp)
                        # attended
                        for t in range(2):
                            for c2 in range(2):
                                nc.tensor.matmul(
                                    attp[c2],
                                    vT[jh][:, bass.ds(jloc[t] * C
                                                      + c2 * 128, 128)],
                                    est[:, t, :],
                                    start=(g == 0 and t == 0),
                                    stop=(g == NG - 1 and t == 1))
                        # r: 2 ones-matmuls col-packed into one PE slot
                        for t in range(2):
                            nc.tensor.matmul(
                                rp[32 * t:32 * t + 1, :], ones_bf,
                                est[:, t, :],
                                start=(g == 0), stop=(g == NG - 1),
                                tile_position=(0, 32 * t))
                    # fold r strips (DMA accumulate), recip, bcast, normalize
                    r_sb = attn_sb.tile([128, 512], F32, tag="rsb", bufs=2,
                                        name="r_sb")
                    nc.vector.tensor_copy(r_sb, rp)
                    rsum = attn_sb.tile([1, 512], F32, tag="rsum",
                                        bufs=2, name="rsum")
                    nc.gpsimd.dma_start(out=rsum, in_=r_sb[0:1, :])
                    nc.gpsimd.dma_start(out=rsum, in_=r_sb[32:33, :],
                                        accum_op=mybir.AluOpType.add)
                    rr = attn_sb.tile([1, 512], F32, tag="rr", bufs=2,
                                      name="rr")
                    nc.vector.reciprocal(rr, rsum)
                    rr_d = attn_dram.tile([1, 512], F32, tag="rrd",
                                          name="rr_d")
                    nc.sync.dma_start(out=rr_d, in_=rr)
                    rrb = attn_sb.tile([128, 512], F32, tag="rrb", bufs=2,
                                       name="rrb")
                    nc.gpsimd.dma_start(out=rrb, in_=_bcast_ap(rr_d, 128))
                    for c2 in range(2):
                        nc.vector.tensor_mul(att_sb[br][c2][:, isl],
                                             attp[c2], rrb)
                    # fused combine for this query block
                    outp = ps_r.tile([1, 512], F32, tag="rp", name="outp")
                    for c2 in range(2):
                        cp = ps_att.tile([128, 512], F32, tag="attp",
                                         bufs=3, name="cp")
                        for kc in range(2):
                            nc.tensor.matmul(
                                cp, wcx_sb[:, kc, bass.ts(c2, 128)],
                                x1i_sb[kc][:, isl],
                                start=(kc == 0), stop=False)
                        for kc in range(2):
                            nc.tensor.matmul(
                                cp, wca_sb[:, kc, bass.ts(c2, 128)],
                                att_sb[br][kc][:, isl],
                                start=False, stop=(kc == 1))
                        absb = attn_sb.tile([128, 512], BF16, tag="absb",
                                            bufs=4, name="absb")
                        nc.scalar.activation(absb, cp, AF.Abs,
                                             bias=bce_sb[:, c2:c2 + 1])
                        nc.tensor.matmul(outp, ones_bf, absb,
                                         start=(c2 == 0), stop=(c2 == 1))
                    osb = attn_sb.tile([1, 512], F32, tag="osb", bufs=2,
                                       name="osb")
                    nc.vector.tensor_copy(osb, outp)
                    nc.sync.dma_start(out=out_d[br:br + 1, isl], in_=osb)


_NC_CACHE = {}


def _get_nc():
    if "nc" not in _NC_CACHE:
        nc = bacc.Bacc("TRN2", debug=False, enable_asserts=False,
                       target_bir_lowering=False, enable_partition_id=False)
        with tile.TileContext(nc) as tc:
            build_program(nc, tc)
        nc.compile()
        _NC_CACHE["nc"] = nc
    return _NC_CACHE["nc"]


def host_inputs(x1, x2, Wq, bq, Wk, bk, Wv, bv, Wc, bc):
    """Build the 8 per-core input maps (host-side sharding/layout only)."""
    f = np.float32
    x1 = np.asarray(x1, f); x2 = np.asarray(x2, f)
    Wq = np.asarray(Wq, f); bq = np.asarray(bq, f)
    Wk = np.asarray(Wk, f)
    Wv = np.asarray(Wv, f); bv = np.asarray(bv, f)
    Wc = np.asarray(Wc, f); bc = np.asarray(bc, f)

    # 4x row-replicated q/k projection weights -> q4/k4 [128, n] layouts
    Wq4 = np.tile(Wq, (4, 1))            # [128, 256]
    Wk4 = np.tile(Wk, (4, 1))
    wqt = np.ascontiguousarray(Wq4.T.reshape(2, 128, 128))
    wkt = np.ascontiguousarray(Wk4.T.reshape(2, 128, 128))
    bq4 = np.tile(bq, 4).reshape(128, 1).copy()
    wvt = np.ascontiguousarray(Wv.T.reshape(2, 128, C))
    WcT = np.ascontiguousarray(Wc.T)     # [512, 256]
    wctx = WcT[:C].reshape(2, 128, C).copy()
    wcta = WcT[C:].reshape(2, 128, C).astype(ml_dtypes.bfloat16)
    bce = (bc + Wc[:, C:] @ bv).reshape(2, 128).T.copy()   # [128, 2]

    in_maps = []
    for core in range(NCORES):
        b, h = divmod(core, 2)
        x1f = x1[b].reshape(C, N).reshape(2, 128, N)
        x2f = x2[b].reshape(C, N).reshape(2, 128, N)
        in_maps.append({
            "x1f": np.ascontiguousarray(x1f),
            "x2f": np.ascontiguousarray(x2f),
            "x1i": np.ascontiguousarray(x1f[:, :, h * IH:(h + 1) * IH]),
            "x2i": np.ascontiguousarray(x2f[:, :, h * IH:(h + 1) * IH]),
            "wqt": wqt, "wkt": wkt, "wvt": wvt,
            "wctx": wctx, "wcta": wcta,
            "bq": bq4, "bce": bce,
        })
    return in_maps


def assemble(results):
    """results: list of 8 dicts with 'out' [2, IH] -> (out1, out2) full."""
    outs = []
    for row in range(2):
        full = np.empty((B, 1, HH, WW), np.float32)
        for b in range(B):
            half0 = results[2 * b]["out"][row]
            half1 = results[2 * b + 1]["out"][row]
            full[b, 0] = np.concatenate([half0, half1]).reshape(HH, WW)
        outs.append(full)
    return outs[0], outs[1]


def kernel(x1, x2, Wq, bq, Wk, bk, Wv, bv, Wc, bc):
    in_maps = host_inputs(x1, x2, Wq, bq, Wk, bk, Wv, bv, Wc, bc)
    nc = _get_nc()
    res = run_bass_kernel_spmd(nc, in_maps, core_ids=list(range(NCORES)))
    return assemble(res.results)



# revision 10
# speedup vs baseline: 1.4497x; 1.4497x over previous
"""Trainium2 Bass kernel for nn_CrossAttention (B=4, C=256, H=W=64).

Sharding: 8 cores = (batch b, query-half h). Host permutes each core's
channel-flattened inputs so the core's own query half occupies columns
0..IH-1 (softmax/attended sums are j-order invariant), letting the
query slice and combine input be SBUF views of the full feature load
(8 MB/core of input DMA instead of 12 MB).

Per core, per branch:
  q = Wq x_i + bq        [32, 2048] stored 4x row-replicated as q4 [128, 2048]
  k = Wk x_f             [32, 4096] 4x row-replicated (bk dropped:
                         constant-in-j shift is softmax-invariant)
  vT = (Wv x_f)^T        [4096, 256] bf16  (bv folded into bc_eff on host)
  S^T[j, i] = k_j . q_i  scores in j-major layout; FOUR K=32 score matmuls
                         run concurrently in PE row strips (tile_position
                         0/32/64/96) into one 4-bank PSUM tile
  E = exp(S^T) bf16      one ACT op per 4-chunk group ([128, 2048])
  r[i] = sum_j E[j, i]   ones-matmuls col-packed 4-per-slot at M=1;
                         strip fold + 128-partition broadcast via one
                         mask matmul; 1/r = exp(-ln r) on ACT (the
                         natural_log_exp table set holds exp AND ln, so
                         no table reloads)
  att[c, i] = sum_j vT[j, c] E[j, i] / r[i]
  comb = Wc [x_i; att] + bc_eff ; out[i] = sum_c |comb|  (abs on DVE via
                         tensor_scalar add+abs_max; ones-matmul reduce)
Pipelining: attended/r matmuls for group g issue after scores/exp of
group g+1 so the PE never waits on exp; the denominator fold, combine,
and output of each 512-query block are deferred into the next block's
group stream (slots 1-5) so the softmax-normalize chain (copy, mask
matmul, ln, exp, mul) runs entirely in the shadow of PE work.  attp is
freed early by copying raw attended sums to SBUF before normalizing.
PSUM: 4 score banks + 2 attended accumulators + 1 r bank + 1 rotating
aux bank (r-broadcast / combine / output) = 8.  No DRAM round trips
between phases -> no HAM re-throttle windows.
"""

import numpy as np
import ml_dtypes

import concourse.bass as bass
import concourse.bacc as bacc
import concourse.tile as tile
import concourse.mybir as mybir
from concourse.bass_utils import run_bass_kernel_spmd

B, C, HH, WW = 4, 256, 64, 64
N = HH * WW          # 4096
CQK = 32
IH = N // 2          # 2048 query rows per core
NCORES = 8
NJC = N // 128       # 32 key-dim 128-chunks
NDG = NJC // 4       # 8 groups of 4 key-chunks
NIB = IH // 512      # 4 query blocks per branch

F32 = mybir.dt.float32
F32R = mybir.dt.float32r
BF16 = mybir.dt.bfloat16
AF = mybir.ActivationFunctionType
ALU = mybir.AluOpType


def build_program(nc, tc):
    # ---- DRAM I/O ------------------------------------------------------
    dram = {}
    for name, shape, dt in [
        ("x1f", [2, 128, N], F32R), ("x2f", [2, 128, N], F32R),
        ("wqt", [2, 128, 128], F32R), ("wkt", [2, 128, 128], F32R),
        ("wvt", [2, 128, C], F32R),
        ("wctx", [2, 128, C], F32R), ("wcta", [2, 128, C], BF16),
        ("bq", [128, 1], F32), ("bce", [128, 2], F32),
        ("mask4", [128, 128], F32R),
    ]:
        dram[name] = nc.dram_tensor(name, shape, dt, kind="ExternalInput").ap()
    out_d = nc.dram_tensor("out", [2, IH], F32, kind="ExternalOutput").ap()

    import contextlib
    with contextlib.ExitStack() as ctx:
        persist = ctx.enter_context(tc.tile_pool(name="persist", bufs=1))

        wq_sb = persist.tile([128, 2, 128], F32R, tag="wq")
        wk_sb = persist.tile([128, 2, 128], F32R, tag="wk")
        wv_sb = persist.tile([128, 2, C], F32R, tag="wv")
        wcx_sb = persist.tile([128, 2, C], F32R, tag="wcx")
        wca_sb = persist.tile([128, 2, C], BF16, tag="wca")
        bq_sb = persist.tile([128, 1], F32, tag="bq")
        bce_sb = persist.tile([128, 2], F32, tag="bce")
        ones_bf = persist.tile([128, 1], BF16, tag="ones")
        mask4_sb = persist.tile([128, 128], F32R, tag="mask4")

        for w, t in [("wqt", wq_sb), ("wkt", wk_sb), ("wvt", wv_sb),
                     ("wctx", wcx_sb), ("wcta", wca_sb)]:
            for kc in range(2):
                nc.sync.dma_start(out=t[:, kc, :], in_=dram[w][kc])
        nc.sync.dma_start(out=bq_sb, in_=dram["bq"])
        nc.sync.dma_start(out=bce_sb, in_=dram["bce"])
        nc.sync.dma_start(out=mask4_sb, in_=dram["mask4"])
        nc.vector.memset(ones_bf, 1.0)

        # x1 features persist (query slice + combine input are views)
        x1f_sb = persist.tile([128, 2, N], F32R, tag="x1f")

        # projection outputs; k4/vT split in j-halves
        q4_sb = [persist.tile([128, IH], F32R, tag=f"q{i}", name=f"q{i}")
                 for i in range(2)]
        k4_sb = [[persist.tile([128, N // 2], F32R, tag=f"k{i}{h}",
                               name=f"k{i}{h}") for h in range(2)]
                 for i in range(2)]
        vT_sb = [[persist.tile([128, (NJC // 2) * C], BF16, tag=f"vt{i}{h}",
                               name=f"vt{i}{h}") for h in range(2)]
                 for i in range(2)]

        # ---- phase 1: projections -------------------------------------
        with tc.tile_pool(name="proj_sb", bufs=2) as proj_sb, \
             tc.tile_pool(name="ps_kq", bufs=1, space="PSUM") as ps_kq, \
             tc.tile_pool(name="ps_vt", bufs=1, space="PSUM") as ps_vt:

            def k_proj(xf_kc, dst, par):
                for jb in range(4):
                    sl = bass.ts(jb, 512)
                    kp = ps_kq.tile([128, 512], F32, tag="kq", bufs=3,
                                    name="kp")
                    for kc in range(2):
                        nc.tensor.matmul(kp, wk_sb[:, kc, :], xf_kc[kc][:, sl],
                                         start=(kc == 0), stop=(kc == 1))
                    if (jb + par) % 2 == 0:
                        nc.scalar.activation(dst[:, sl], kp, AF.Copy)
                    else:
                        nc.vector.tensor_copy(dst[:, sl], kp)

            def q_proj(xf_kc, dst):
                for qb in range(4):
                    sl = bass.ts(qb, 512)
                    qp = ps_kq.tile([128, 512], F32, tag="kq", bufs=3,
                                    name="qp")
                    for kc in range(2):
                        nc.tensor.matmul(qp, wq_sb[:, kc, :], xf_kc[kc][:, sl],
                                         start=(kc == 0), stop=(kc == 1))
                    nc.scalar.activation(dst[:, sl], qp, AF.Identity,
                                         bias=bq_sb)

            def vt_proj(xf_kc, dst):
                for g in range(4):
                    vtp = ps_vt.tile([128, 4, C], F32, tag="vt", bufs=2,
                                     name="vtp")
                    for s in range(4):
                        jsub = g * 4 + s
                        for kc in range(2):
                            nc.tensor.matmul(
                                vtp[:, s, :],
                                xf_kc[kc][:, bass.ts(jsub, 128)],
                                wv_sb[:, kc, :],
                                start=(kc == 0), stop=(kc == 1))
                    nc.vector.tensor_copy(
                        dst[:, bass.ds(g * 4 * C, 4 * C)],
                        vtp.rearrange("p a c -> p (a c)"))

            # x2 first: k2, q2, vT2 (branch 0 consumes vT2)
            for jh in range(2):
                xf_t = proj_sb.tile([128, 2, IH], F32R, tag="xf", name="xf")
                for kc in range(2):
                    nc.sync.dma_start(
                        out=xf_t[:, kc, :],
                        in_=dram["x2f"][kc][:, jh * IH:(jh + 1) * IH])
                xf_kc = [xf_t[:, kc, :] for kc in range(2)]
                k_proj(xf_kc, k4_sb[1][jh], jh)
                if jh == 0:
                    q_proj(xf_kc, q4_sb[1])
                vt_proj(xf_kc, vT_sb[1][jh])

            # x1: k1 + q1 first so branch 0 can start; vT1 last
            x1_kc = [[x1f_sb[:, kc, jh * IH:(jh + 1) * IH] for kc in range(2)]
                     for jh in range(2)]
            for jh in range(2):
                for kc in range(2):
                    nc.sync.dma_start(
                        out=x1f_sb[:, kc, jh * IH:(jh + 1) * IH],
                        in_=dram["x1f"][kc][:, jh * IH:(jh + 1) * IH])
                k_proj(x1_kc[jh], k4_sb[0][jh], jh)
                if jh == 0:
                    q_proj(x1_kc[jh], q4_sb[0])
            for jh in range(2):
                vt_proj(x1_kc[jh], vT_sb[0][jh])

        x1i_kc = [x1f_sb[:, kc, 0:IH] for kc in range(2)]

        # ---- phase 2: attention + deferred combine --------------------
        with tc.tile_pool(name="attn_sb", bufs=1) as asb, \
             tc.tile_pool(name="ps_st", bufs=1, space="PSUM") as ps_st, \
             tc.tile_pool(name="ps_att", bufs=1, space="PSUM") as ps_att, \
             tc.tile_pool(name="ps_r", bufs=1, space="PSUM") as ps_r, \
             tc.tile_pool(name="ps_aux", bufs=1, space="PSUM") as ps_aux:

            blocks = [(br, ib) for br in range(2) for ib in range(NIB)]

            def emit_block(br, ib, pending):
                """Emit one 512-query block; interleave `pending` (tail +
                combine hooks of the previous block, keyed by group slot).
                Returns this block's pending dict."""
                isl = bass.ts(ib, 512)
                q4, k4, vT = q4_sb[br], k4_sb[br], vT_sb[1 - br]

                attp = [ps_att.tile([128, 512], F32, tag="attp", bufs=2,
                                    name=f"attp{c2}") for c2 in range(2)]
                rp = ps_r.tile([128, 512], F32, tag="rp", bufs=1, name="rp")
                nc.vector.memset(rp, 0.0)

                def scores(dg):
                    stp = ps_st.tile([128, 4, 512], F32, tag="stp", bufs=1,
                                     name="stp")
                    for u in range(4):
                        jc = dg * 4 + u
                        jh, jloc = jc // 16, jc % 16
                        nc.tensor.matmul(
                            stp[:, u, :],
                            k4[jh][32 * u:32 * (u + 1), bass.ts(jloc, 128)],
                            q4[32 * u:32 * (u + 1), isl],
                            start=True, stop=True, tile_position=(32 * u, 0))
                    est = asb.tile([128, 4, 512], BF16, tag="est", bufs=3,
                                   name="est")
                    nc.scalar.activation(
                        est.rearrange("p a n -> p (a n)"),
                        stp.rearrange("p a n -> p (a n)"), AF.Exp)
                    return est

                def attended(dg, est):
                    for u in range(4):
                        jc = dg * 4 + u
                        jh, jloc = jc // 16, jc % 16
                        for c2 in range(2):
                            nc.tensor.matmul(
                                attp[c2],
                                vT[jh][:, bass.ds(jloc * C + c2 * 128, 128)],
                                est[:, u, :],
                                start=(dg == 0 and u == 0),
                                stop=(dg == NDG - 1 and u == 3))
                    for u in range(4):
                        nc.tensor.matmul(
                            rp[32 * u:32 * u + 1, :], ones_bf, est[:, u, :],
                            start=(dg == 0), stop=(dg == NDG - 1),
                            tile_position=(0, 32 * u))

                est_prev = None
                for dg in range(NDG):
                    est = scores(dg)
                    if dg > 0:
                        attended(dg - 1, est_prev)
                    est_prev = est
                    if pending is not None:
                        for fn in pending.get(dg, ()):
                            fn()
                attended(NDG - 1, est_prev)

                # --- tail: free attp, start denominator chain ----------
                att_raw = [asb.tile([128, 512], BF16, tag="attraw", bufs=4,
                                    name=f"attraw{c2}") for c2 in range(2)]
                for c2 in range(2):
                    nc.vector.tensor_copy(att_raw[c2], attp[c2])
                r_sb = asb.tile([128, 512], F32R, tag="rsb", bufs=2,
                                name="r_sb")
                nc.vector.tensor_copy(r_sb, rp)

                # --- deferred into next block's group slots ------------
                att_n = [asb.tile([128, 512], BF16, tag="attsb", bufs=4,
                                  name=f"attn{c2}") for c2 in range(2)]
                lnr = asb.tile([128, 512], F32, tag="lnr", bufs=2, name="lnr")
                rinv = asb.tile([128, 512], F32, tag="rinv", bufs=2,
                                name="rinv")

                def t_fold():
                    rb = ps_aux.tile([128, 512], F32, tag="aux", bufs=1,
                                     name="rb")
                    nc.tensor.matmul(rb, mask4_sb, r_sb, start=True, stop=True)
                    nc.scalar.activation(lnr, rb, AF.Ln)
                    nc.scalar.activation(rinv, lnr, AF.Exp, scale=-1.0)

                def t_norm():
                    for c2 in range(2):
                        nc.vector.tensor_mul(att_n[c2], att_raw[c2], rinv)

                absb = [asb.tile([128, 512], BF16, tag="absb", bufs=4,
                                 name=f"absb{c2}") for c2 in range(2)]
                cps = []

                def t_comb(c2):
                    cp = ps_aux.tile([128, 512], F32, tag="aux", bufs=1,
                                     name=f"cp{c2}")
                    cps.append(cp)
                    for kc in range(2):
                        nc.tensor.matmul(
                            cp, wcx_sb[:, kc, bass.ts(c2, 128)],
                            x1i_kc[kc][:, isl],
                            start=(kc == 0), stop=False)
                    for kc in range(2):
                        nc.tensor.matmul(
                            cp, wca_sb[:, kc, bass.ts(c2, 128)],
                            att_n[kc],
                            start=False, stop=(kc == 1))
                    nc.scalar.activation(absb[c2], cp, AF.Abs,
                                         bias=bce_sb[:, c2:c2 + 1])

                def t_out():
                    outp = ps_aux.tile([128, 512], F32, tag="aux", bufs=1,
                                       name="outp")
                    for c2 in range(2):
                        nc.tensor.matmul(outp[0:1, :], ones_bf, absb[c2],
                                         start=(c2 == 0), stop=(c2 == 1))
                    osb = asb.tile([1, 512], F32, tag="osb", bufs=2,
                                   name="osb")
                    nc.vector.tensor_copy(osb, outp[0:1, :])
                    nc.sync.dma_start(out=out_d[br:br + 1, isl], in_=osb)

                return {1: (t_fold,), 2: (t_norm,),
                        4: (lambda: t_comb(0),), 5: (lambda: t_comb(1),),
                        6: (t_out,)}

            pending = None
            for br, ib in blocks:
                pending = emit_block(br, ib, pending)
            # epilogue: flush the last block's tail at the end
            for step in sorted(pending):
                for fn in pending[step]:
                    fn()


_NC_CACHE = {}


def _get_nc():
    if "nc" not in _NC_CACHE:
        nc = bacc.Bacc("TRN2", debug=False, enable_asserts=False,
                       target_bir_lowering=False, enable_partition_id=False)
        with tile.TileContext(nc) as tc:
            build_program(nc, tc)
        nc.compile()
        _NC_CACHE["nc"] = nc
    return _NC_CACHE["nc"]


def host_inputs(x1, x2, Wq, bq, Wk, bk, Wv, bv, Wc, bc):
    """Build the 8 per-core input maps (host-side sharding/layout only)."""
    f = np.float32
    x1 = np.asarray(x1, f); x2 = np.asarray(x2, f)
    Wq = np.asarray(Wq, f); bq = np.asarray(bq, f)
    Wk = np.asarray(Wk, f)
    Wv = np.asarray(Wv, f); bv = np.asarray(bv, f)
    Wc = np.asarray(Wc, f); bc = np.asarray(bc, f)

    # 4x row-replicated q/k projection weights
    Wq4 = np.tile(Wq, (4, 1))            # [128, 256]
    Wk4 = np.tile(Wk, (4, 1))
    wqt = np.ascontiguousarray(Wq4.T.reshape(2, 128, 128))
    wkt = np.ascontiguousarray(Wk4.T.reshape(2, 128, 128))
    bq4 = np.tile(bq, 4).reshape(128, 1).copy()
    wvt = np.ascontiguousarray(Wv.T.reshape(2, 128, C))
    WcT = np.ascontiguousarray(Wc.T)     # [512, 256]
    wctx = WcT[:C].reshape(2, 128, C).copy()
    wcta = WcT[C:].reshape(2, 128, C).astype(ml_dtypes.bfloat16)
    bce = (bc + Wc[:, C:] @ bv).reshape(2, 128).T.copy()   # [128, 2]
    mask4 = np.zeros((128, 128), np.float32)
    mask4[0::32, :] = 1.0        # fold rows 0/32/64/96 -> all partitions

    in_maps = []
    for core in range(NCORES):
        b, h = divmod(core, 2)
        x1f = x1[b].reshape(C, N).reshape(2, 128, N)
        x2f = x2[b].reshape(C, N).reshape(2, 128, N)
        if h == 1:   # rotate so this core's query half is columns 0..IH-1
            x1f = np.concatenate([x1f[:, :, IH:], x1f[:, :, :IH]], axis=2)
            x2f = np.concatenate([x2f[:, :, IH:], x2f[:, :, :IH]], axis=2)
        in_maps.append({
            "x1f": np.ascontiguousarray(x1f),
            "x2f": np.ascontiguousarray(x2f),
            "wqt": wqt, "wkt": wkt, "wvt": wvt,
            "wctx": wctx, "wcta": wcta,
            "bq": bq4, "bce": bce, "mask4": mask4,
        })
    return in_maps


def assemble(results):
    """results: list of 8 dicts with 'out' [2, IH] -> (out1, out2) full."""
    outs = []
    for row in range(2):
        full = np.empty((B, 1, HH, WW), np.float32)
        for b in range(B):
            half0 = results[2 * b]["out"][row]
            half1 = results[2 * b + 1]["out"][row]
            full[b, 0] = np.concatenate([half0, half1]).reshape(HH, WW)
        outs.append(full)
    return outs[0], outs[1]


def kernel(x1, x2, Wq, bq, Wk, bk, Wv, bv, Wc, bc):
    in_maps = host_inputs(x1, x2, Wq, bq, Wk, bk, Wv, bv, Wc, bc)
    nc = _get_nc()
    res = run_bass_kernel_spmd(nc, in_maps, core_ids=list(range(NCORES)))
    return assemble(res.results)


# revision 11
# speedup vs baseline: 1.5949x; 1.1001x over previous
"""Trainium2 Bass kernel for nn_CrossAttention (B=4, C=256, H=W=64).

Sharding: 8 cores = (batch b, query-half h). Host permutes each core's
channel-flattened inputs so the core's own query half occupies columns
0..IH-1 (softmax/attended sums are j-order invariant), letting the
query slice and combine input be SBUF views of the full feature load
(8 MB/core of input DMA instead of 12 MB).

Per core, per branch:
  q = Wq x_i + bq        [32, 2048] stored 4x row-replicated as q4 [128, 2048]
  k = Wk x_f             [32, 4096] 4x row-replicated (bk dropped:
                         constant-in-j shift is softmax-invariant)
  vT = (Wv x_f)^T        [4096, 256] bf16  (bv folded into bc_eff on host)
  S^T[j, i] = k_j . q_i  scores in j-major layout; FOUR K=32 score matmuls
                         run concurrently in PE row strips (tile_position
                         0/32/64/96) into one 4-bank PSUM tile
  E = exp(S^T) bf16      one ACT op per 4-chunk group ([128, 2048])
  r[i] = sum_j E[j, i]   ones-matmuls col-packed 4-per-slot at M=1;
                         strip fold + 128-partition broadcast via one
                         mask matmul; 1/r = exp(-ln r) on ACT (the
                         natural_log_exp table set holds exp AND ln, so
                         no table reloads)
  att[c, i] = sum_j vT[j, c] E[j, i] / r[i]
  comb = Wc [x_i; att] + bc_eff ; out[i] = sum_c |comb|  (abs on DVE via
                         tensor_scalar add+abs_max; ones-matmul reduce)
Pipelining: attended/r matmuls for group g issue after scores/exp of
group g+1 so the PE never waits on exp; the denominator fold, combine,
and output of each 512-query block are deferred into the next block's
group stream (slots 1-5) so the softmax-normalize chain (copy, mask
matmul, ln, exp, mul) runs entirely in the shadow of PE work.  attp is
freed early by copying raw attended sums to SBUF before normalizing.
PSUM: 4 score banks + 2 attended accumulators + 1 r bank + 1 rotating
aux bank (r-broadcast / combine / output) = 8.  No DRAM round trips
between phases -> no HAM re-throttle windows.
"""

import numpy as np
import ml_dtypes

import concourse.bass as bass
import concourse.bacc as bacc
import concourse.tile as tile
import concourse.mybir as mybir
from concourse.bass_utils import run_bass_kernel_spmd

B, C, HH, WW = 4, 256, 64, 64
N = HH * WW          # 4096
CQK = 32
IH = N // 2          # 2048 query rows per core
NCORES = 8
NJC = N // 128       # 32 key-dim 128-chunks
NDG = NJC // 4       # 8 groups of 4 key-chunks
NIB = IH // 512      # 4 query blocks per branch

F32 = mybir.dt.float32
F32R = mybir.dt.float32r
BF16 = mybir.dt.bfloat16
AF = mybir.ActivationFunctionType
ALU = mybir.AluOpType


def build_program(nc, tc):
    # ---- DRAM I/O ------------------------------------------------------
    dram = {}
    for name, shape, dt in [
        ("x1f", [2, 128, N], F32R), ("x2f", [2, 128, N], F32R),
        ("wqt", [2, 128, 128], F32R), ("wkt", [2, 128, 128], F32R),
        ("wvt", [2, 128, C], F32R),
        ("wctx", [2, 128, C], F32R), ("wcta", [2, 128, C], BF16),
        ("bq", [128, 1], F32), ("bce", [128, 2], F32),
        ("mask4", [128, 128], F32R),
    ]:
        dram[name] = nc.dram_tensor(name, shape, dt, kind="ExternalInput").ap()
    out_d = nc.dram_tensor("out", [2, IH], F32, kind="ExternalOutput").ap()

    import contextlib
    with contextlib.ExitStack() as ctx:
        persist = ctx.enter_context(tc.tile_pool(name="persist", bufs=1))

        wq_sb = persist.tile([128, 2, 128], F32R, tag="wq")
        wk_sb = persist.tile([128, 2, 128], F32R, tag="wk")
        wv_sb = persist.tile([128, 2, C], F32R, tag="wv")
        wcx_sb = persist.tile([128, 2, C], F32R, tag="wcx")
        wca_sb = persist.tile([128, 2, C], BF16, tag="wca")
        bq_sb = persist.tile([128, 1], F32, tag="bq")
        bce_sb = persist.tile([128, 2], F32, tag="bce")
        ones_bf = persist.tile([128, 1], BF16, tag="ones")
        mask4_sb = persist.tile([128, 128], F32R, tag="mask4")

        for w, t in [("wqt", wq_sb), ("wkt", wk_sb), ("wvt", wv_sb),
                     ("wctx", wcx_sb), ("wcta", wca_sb)]:
            for kc in range(2):
                nc.sync.dma_start(out=t[:, kc, :], in_=dram[w][kc])
        nc.sync.dma_start(out=bq_sb, in_=dram["bq"])
        nc.sync.dma_start(out=bce_sb, in_=dram["bce"])
        nc.sync.dma_start(out=mask4_sb, in_=dram["mask4"])
        nc.vector.memset(ones_bf, 1.0)

        # x1 features persist (query slice + combine input are views)
        x1f_sb = persist.tile([128, 2, N], F32R, tag="x1f")

        # projection outputs; k4/vT split in j-halves
        q4_sb = [persist.tile([128, IH], F32R, tag=f"q{i}", name=f"q{i}")
                 for i in range(2)]
        k4_sb = [[persist.tile([128, N // 2], F32R, tag=f"k{i}{h}",
                               name=f"k{i}{h}") for h in range(2)]
                 for i in range(2)]
        vT_sb = [[persist.tile([128, (NJC // 2) * C], BF16, tag=f"vt{i}{h}",
                               name=f"vt{i}{h}") for h in range(2)]
                 for i in range(2)]

        # ---- phase 1: projections -------------------------------------
        with tc.tile_pool(name="proj_sb", bufs=2) as proj_sb, \
             tc.tile_pool(name="ps_kq", bufs=1, space="PSUM") as ps_kq, \
             tc.tile_pool(name="ps_vt", bufs=1, space="PSUM") as ps_vt:

            def k_proj(xf_kc, dst, par):
                for jb in range(4):
                    sl = bass.ts(jb, 512)
                    kp = ps_kq.tile([128, 512], F32, tag="kq", bufs=3,
                                    name="kp")
                    for kc in range(2):
                        nc.tensor.matmul(kp, wk_sb[:, kc, :], xf_kc[kc][:, sl],
                                         start=(kc == 0), stop=(kc == 1))
                    if (jb + par) % 2 == 0:
                        nc.scalar.activation(dst[:, sl], kp, AF.Copy)
                    else:
                        nc.vector.tensor_copy(dst[:, sl], kp)

            def q_proj(xf_kc, dst):
                for qb in range(4):
                    sl = bass.ts(qb, 512)
                    qp = ps_kq.tile([128, 512], F32, tag="kq", bufs=3,
                                    name="qp")
                    for kc in range(2):
                        nc.tensor.matmul(qp, wq_sb[:, kc, :], xf_kc[kc][:, sl],
                                         start=(kc == 0), stop=(kc == 1))
                    nc.scalar.activation(dst[:, sl], qp, AF.Identity,
                                         bias=bq_sb)

            def vt_proj(xf_kc, dst):
                for g in range(4):
                    vtp = ps_vt.tile([128, 4, C], F32, tag="vt", bufs=2,
                                     name="vtp")
                    for s in range(4):
                        jsub = g * 4 + s
                        for kc in range(2):
                            nc.tensor.matmul(
                                vtp[:, s, :],
                                xf_kc[kc][:, bass.ts(jsub, 128)],
                                wv_sb[:, kc, :],
                                start=(kc == 0), stop=(kc == 1))
                    nc.vector.tensor_copy(
                        dst[:, bass.ds(g * 4 * C, 4 * C)],
                        vtp.rearrange("p a c -> p (a c)"))

            # x2 first: k2, q2, vT2 (branch 0 consumes vT2)
            for jh in range(2):
                xf_t = proj_sb.tile([128, 2, IH], F32R, tag="xf", name="xf")
                for kc in range(2):
                    nc.sync.dma_start(
                        out=xf_t[:, kc, :],
                        in_=dram["x2f"][kc][:, jh * IH:(jh + 1) * IH])
                xf_kc = [xf_t[:, kc, :] for kc in range(2)]
                k_proj(xf_kc, k4_sb[1][jh], jh)
                if jh == 0:
                    q_proj(xf_kc, q4_sb[1])
                vt_proj(xf_kc, vT_sb[1][jh])

            # x1: k1 + q1 first so branch 0 can start; vT1 last
            x1_kc = [[x1f_sb[:, kc, jh * IH:(jh + 1) * IH] for kc in range(2)]
                     for jh in range(2)]
            for jh in range(2):
                for kc in range(2):
                    nc.sync.dma_start(
                        out=x1f_sb[:, kc, jh * IH:(jh + 1) * IH],
                        in_=dram["x1f"][kc][:, jh * IH:(jh + 1) * IH])
                k_proj(x1_kc[jh], k4_sb[0][jh], jh)
                if jh == 0:
                    q_proj(x1_kc[jh], q4_sb[0])
            for jh in range(2):
                vt_proj(x1_kc[jh], vT_sb[0][jh])

        x1i_kc = [x1f_sb[:, kc, 0:IH] for kc in range(2)]

        # ---- phase 2: attention + deferred combine --------------------
        with tc.tile_pool(name="attn_sb", bufs=1) as asb, \
             tc.tile_pool(name="ps_st", bufs=1, space="PSUM") as ps_st, \
             tc.tile_pool(name="ps_att", bufs=1, space="PSUM") as ps_att, \
             tc.tile_pool(name="ps_r", bufs=1, space="PSUM") as ps_r, \
             tc.tile_pool(name="ps_aux", bufs=1, space="PSUM") as ps_aux:

            blocks = [(br, ib) for br in range(2) for ib in range(NIB)]

            def emit_block(br, ib, pending):
                """Emit one 512-query block; interleave `pending` (tail +
                combine hooks of the previous block, keyed by group slot).
                Returns this block's pending dict."""
                isl = bass.ts(ib, 512)
                q4, k4, vT = q4_sb[br], k4_sb[br], vT_sb[1 - br]

                attp = [ps_att.tile([128, 512], F32, tag="attp", bufs=2,
                                    name=f"attp{c2}") for c2 in range(2)]
                rp = ps_r.tile([128, 512], F32, tag="rp", bufs=1, name="rp")
                nc.vector.memset(rp, 0.0)

                def scores(dg):
                    stp = ps_st.tile([128, 4, 512], F32, tag="stp", bufs=1,
                                     name="stp")
                    for u in range(4):
                        jc = dg * 4 + u
                        jh, jloc = jc // 16, jc % 16
                        nc.tensor.matmul(
                            stp[:, u, :],
                            k4[jh][32 * u:32 * (u + 1), bass.ts(jloc, 128)],
                            q4[32 * u:32 * (u + 1), isl],
                            start=True, stop=True, tile_position=(32 * u, 0))
                    est = asb.tile([128, 4, 512], BF16, tag="est", bufs=3,
                                   name="est")
                    nc.scalar.activation(
                        est.rearrange("p a n -> p (a n)"),
                        stp.rearrange("p a n -> p (a n)"), AF.Exp)
                    return est

                def attended(dg, est):
                    for u in range(4):
                        jc = dg * 4 + u
                        jh, jloc = jc // 16, jc % 16
                        for c2 in range(2):
                            nc.tensor.matmul(
                                attp[c2],
                                vT[jh][:, bass.ds(jloc * C + c2 * 128, 128)],
                                est[:, u, :],
                                start=(dg == 0 and u == 0),
                                stop=(dg == NDG - 1 and u == 3))
                    for u in range(4):
                        nc.tensor.matmul(
                            rp[32 * u:32 * u + 1, :], ones_bf, est[:, u, :],
                            start=(dg == 0), stop=(dg == NDG - 1),
                            tile_position=(0, 32 * u))

                est_prev = None
                for dg in range(NDG):
                    est = scores(dg)
                    if dg > 0:
                        attended(dg - 1, est_prev)
                    est_prev = est
                    if pending is not None:
                        for fn in pending.get(dg, ()):
                            fn()
                attended(NDG - 1, est_prev)

                # --- tail: free attp, start denominator chain ----------
                att_raw = [asb.tile([128, 512], BF16, tag="attraw", bufs=4,
                                    name=f"attraw{c2}") for c2 in range(2)]
                for c2 in range(2):
                    nc.vector.tensor_copy(att_raw[c2], attp[c2])
                r_sb = asb.tile([128, 512], F32R, tag="rsb", bufs=2,
                                name="r_sb")
                nc.vector.tensor_copy(r_sb, rp)

                # --- deferred into next block's group slots ------------
                att_n = [asb.tile([128, 512], BF16, tag="attsb", bufs=4,
                                  name=f"attn{c2}") for c2 in range(2)]
                lnr = asb.tile([128, 512], F32, tag="lnr", bufs=2, name="lnr")
                rinv = asb.tile([128, 512], F32, tag="rinv", bufs=2,
                                name="rinv")

                def t_fold():
                    rb = ps_aux.tile([128, 512], F32, tag="aux", bufs=1,
                                     name="rb")
                    nc.tensor.matmul(rb, mask4_sb, r_sb, start=True, stop=True)
                    nc.scalar.activation(lnr, rb, AF.Ln)
                    nc.scalar.activation(rinv, lnr, AF.Exp, scale=-1.0)

                def t_norm():
                    for c2 in range(2):
                        nc.vector.tensor_mul(att_n[c2], att_raw[c2], rinv)

                absb = [asb.tile([128, 512], BF16, tag="absb", bufs=4,
                                 name=f"absb{c2}") for c2 in range(2)]
                cps = []

                def t_comb(c2):
                    cp = ps_aux.tile([128, 512], F32, tag="aux", bufs=1,
                                     name=f"cp{c2}")
                    cps.append(cp)
                    for kc in range(2):
                        nc.tensor.matmul(
                            cp, wcx_sb[:, kc, bass.ts(c2, 128)],
                            x1i_kc[kc][:, isl],
                            start=(kc == 0), stop=False)
                    for kc in range(2):
                        nc.tensor.matmul(
                            cp, wca_sb[:, kc, bass.ts(c2, 128)],
                            att_n[kc],
                            start=False, stop=(kc == 1))
                    nc.scalar.activation(absb[c2], cp, AF.Abs,
                                         bias=bce_sb[:, c2:c2 + 1])

                def t_out():
                    outp = ps_aux.tile([128, 512], F32, tag="aux", bufs=1,
                                       name="outp")
                    for c2 in range(2):
                        nc.tensor.matmul(outp[0:1, :], ones_bf, absb[c2],
                                         start=(c2 == 0), stop=(c2 == 1))
                    osb = asb.tile([1, 512], F32, tag="osb", bufs=2,
                                   name="osb")
                    nc.vector.tensor_copy(osb, outp[0:1, :])
                    nc.sync.dma_start(out=out_d[br:br + 1, isl], in_=osb)

                return {1: (t_fold,), 2: (t_norm,),
                        4: (lambda: t_comb(0),), 5: (lambda: t_comb(1),),
                        6: (t_out,)}

            pending = None
            for br, ib in blocks:
                pending = emit_block(br, ib, pending)
            # epilogue: flush the last block's tail at the end
            for step in sorted(pending):
                for fn in pending[step]:
                    fn()


class _BaccOneActTable(bacc.Bacc):
    """Pin every activation to the natural_log_exp_and_others table set
    (it contains Exp, Ln, Abs, Copy and Identity — everything this kernel
    uses).  The default chooser assigns Exp to exp_and_others and Ln to
    natural_log_exp_and_others, reloading tables twice per block (~2.7us
    each on the Scalar engine).  Set indices are preserved so walrus's
    act_func_set_id remap stays valid."""

    def insert_act_table_loads(self):
        import bass_rust as _br
        from concourse.hw_specs import get_activation_tables
        has_activation = any(
            isinstance(i, mybir.InstActivation)
            for b in self.main_func.blocks
            for i in b.instructions
        )
        if not has_activation:
            return
        keep = "natural_log_exp_and_others"
        tables = [(name, funcs if name == keep else set())
                  for name, funcs in
                  get_activation_tables(self.m.arch).items()]
        _br.insert_act_table_loads(self, tables)


_NC_CACHE = {}


def _get_nc():
    if "nc" not in _NC_CACHE:
        nc = _BaccOneActTable(
            "TRN2", debug=False, enable_asserts=False,
            target_bir_lowering=False, enable_partition_id=False)
        with tile.TileContext(nc) as tc:
            build_program(nc, tc)
        nc.compile()
        _NC_CACHE["nc"] = nc
    return _NC_CACHE["nc"]


def host_inputs(x1, x2, Wq, bq, Wk, bk, Wv, bv, Wc, bc):
    """Build the 8 per-core input maps (host-side sharding/layout only)."""
    f = np.float32
    x1 = np.asarray(x1, f); x2 = np.asarray(x2, f)
    Wq = np.asarray(Wq, f); bq = np.asarray(bq, f)
    Wk = np.asarray(Wk, f)
    Wv = np.asarray(Wv, f); bv = np.asarray(bv, f)
    Wc = np.asarray(Wc, f); bc = np.asarray(bc, f)

    # 4x row-replicated q/k projection weights
    Wq4 = np.tile(Wq, (4, 1))            # [128, 256]
    Wk4 = np.tile(Wk, (4, 1))
    wqt = np.ascontiguousarray(Wq4.T.reshape(2, 128, 128))
    wkt = np.ascontiguousarray(Wk4.T.reshape(2, 128, 128))
    bq4 = np.tile(bq, 4).reshape(128, 1).copy()
    wvt = np.ascontiguousarray(Wv.T.reshape(2, 128, C))
    WcT = np.ascontiguousarray(Wc.T)     # [512, 256]
    wctx = WcT[:C].reshape(2, 128, C).copy()
    wcta = WcT[C:].reshape(2, 128, C).astype(ml_dtypes.bfloat16)
    bce = (bc + Wc[:, C:] @ bv).reshape(2, 128).T.copy()   # [128, 2]
    mask4 = np.zeros((128, 128), np.float32)
    mask4[0::32, :] = 1.0        # fold rows 0/32/64/96 -> all partitions

    in_maps = []
    for core in range(NCORES):
        b, h = divmod(core, 2)
        x1f = x1[b].reshape(C, N).reshape(2, 128, N)
        x2f = x2[b].reshape(C, N).reshape(2, 128, N)
        if h == 1:   # rotate so this core's query half is columns 0..IH-1
            x1f = np.concatenate([x1f[:, :, IH:], x1f[:, :, :IH]], axis=2)
            x2f = np.concatenate([x2f[:, :, IH:], x2f[:, :, :IH]], axis=2)
        in_maps.append({
            "x1f": np.ascontiguousarray(x1f),
            "x2f": np.ascontiguousarray(x2f),
            "wqt": wqt, "wkt": wkt, "wvt": wvt,
            "wctx": wctx, "wcta": wcta,
            "bq": bq4, "bce": bce, "mask4": mask4,
        })
    return in_maps


def assemble(results):
    """results: list of 8 dicts with 'out' [2, IH] -> (out1, out2) full."""
    outs = []
    for row in range(2):
        full = np.empty((B, 1, HH, WW), np.float32)
        for b in range(B):
            half0 = results[2 * b]["out"][row]
            half1 = results[2 * b + 1]["out"][row]
            full[b, 0] = np.concatenate([half0, half1]).reshape(HH, WW)
        outs.append(full)
    return outs[0], outs[1]


def kernel(x1, x2, Wq, bq, Wk, bk, Wv, bv, Wc, bc):
    in_maps = host_inputs(x1, x2, Wq, bq, Wk, bk, Wv, bv, Wc, bc)
    nc = _get_nc()
    res = run_bass_kernel_spmd(nc, in_maps, core_ids=list(range(NCORES)))
    return assemble(res.results)
